# revision 15
# baseline (speedup 1.0000x reference)
"""Associative-embedding loss (push/pull) on 8 TRN2 NeuronCores.

Strategy (pure data parallel, hardcoded):
  - B=32 images, 8 cores -> 4 images per core.
  - Per image only 510 of the 278528 tag rows are needed, so the kernel
    never streams the tags tensor: indirect (SWDGE) DMAs gather the 510
    rows (128 partitions x 4 slots x 4 floats, one slot column per
    instruction) straight from HBM. Gather descriptor work (~10.5 ns per
    row, serialized on GpSimd) is the critical path, so the index load
    and the 16 gathers are issued first and everything else pipelines
    underneath them.
  - Per-person sums are tiny TensorE matmuls of the gathered rows
    against per-image membership matrices with visibility folded in
    (member[slot, person] = vis * (slot//17 == person)), so the matmul
    consumes the gather output directly.
  - pull is reduced to two scalars per image:
      pull = sum_p S2_p*invcv_p - sum_p q_p*valid_p
    (first term: fused dot of squared gathered rows with a host weight
    map that folds vis * invcv, second from q = |mean|^2).
  - Pairwise push term: diff[i,j] = q_i + q_j - 2<m_i, m_j> built by
    three accumulating matmuls per image into a shared [30,120] PSUM
    tile; one exp/mask/reduce finale covers all 4 images at once.
  - Everything that depends only on `keypoints` (visibility, counts,
    pair masks, scales) is precomputed on the host - a few KB per image.

Inputs: tags [32, 278528, 4] f32, keypoints [32, 30, 17, 2] int.
Output: [32, 2] f32 (push, pull) per image.
"""

import numpy as np

import concourse.bacc as bacc
import concourse.bass as bass
import concourse.mybir as mybir
import concourse.tile as tile
from concourse.bass_utils import run_bass_kernel_spmd

B, N, D = 32, 278528, 4
NPERS, NKP = 30, 17
NFLAT = NPERS * NKP          # 510 keypoints per image
KPJ = 4                      # keypoint slots per SBUF partition (128*4=512)
NCORES = 8
IMGS = B // NCORES           # 4 images per core
W = IMGS * NPERS             # 120: merged free width
EPS = 1e-6
X = mybir.AxisListType


def _build_nc():
    nc = bacc.Bacc("TRN2", target_bir_lowering=False, debug=False)
    f32 = mybir.dt.float32
    tags = nc.dram_tensor("tags", [IMGS * N, D], f32, kind="ExternalInput").ap()
    idx = nc.dram_tensor("idx", [128, IMGS * KPJ], mybir.dt.int32, kind="ExternalInput").ap()
    w2 = nc.dram_tensor("w2", [128, IMGS * KPJ * D], f32, kind="ExternalInput").ap()
    member = nc.dram_tensor("member", [128, KPJ * NPERS], f32, kind="ExternalInput").ap()
    mvis = nc.dram_tensor("mvis", [128, IMGS * KPJ * D], f32, kind="ExternalInput").ap()
    inv4 = nc.dram_tensor("inv4", [4, W], f32, kind="ExternalInput").ap()
    vrow = nc.dram_tensor("vrow", [1, W], f32, kind="ExternalInput").ap()
    hmask = nc.dram_tensor("hmask", [NPERS, W], f32, kind="ExternalInput").ap()
    scales = nc.dram_tensor("scales", [1, 2 * IMGS], f32, kind="ExternalInput").ap()
    out = nc.dram_tensor("out", [1, IMGS * 2], f32, kind="ExternalOutput").ap()

    with tile.TileContext(nc) as tc:
        with (
            tc.tile_pool(name="const", bufs=1) as cpool,
            tc.tile_pool(name="work", bufs=4) as wpool,
            tc.tile_pool(name="merge", bufs=1) as mpool,
            tc.tile_pool(name="psum", bufs=1, space="PSUM") as ppool,
        ):
            # critical path first: index loads on GpSimd, then the 16 gathers
            idx_t = cpool.tile([128, IMGS * KPJ], mybir.dt.int32)
            nc.gpsimd.dma_start(idx_t[:, 0:1], idx[:, 0:1])
            nc.gpsimd.dma_start(idx_t[:, 1:KPJ], idx[:, 1:KPJ])
            g_ts = []
            for b in range(IMGS):
                g_t = wpool.tile([128, KPJ * D], f32, tag="g")
                g_ts.append(g_t)
                for j in range(KPJ):
                    nc.gpsimd.indirect_dma_start(
                        out=g_t[:, j * D:(j + 1) * D],
                        out_offset=None,
                        in_=tags,
                        in_offset=bass.IndirectOffsetOnAxis(
                            ap=idx_t[:, b * KPJ + j:b * KPJ + j + 1], axis=0
                        ),
                    )
                if b == 0:
                    nc.gpsimd.dma_start(idx_t[:, KPJ:], idx[:, KPJ:])

            member_t = cpool.tile([128, KPJ * NPERS], f32)
            nc.sync.dma_start(member_t[:], member)
            mvis_t = cpool.tile([128, IMGS * KPJ * D], f32)
            nc.sync.dma_start(mvis_t[:], mvis)
            w2_t = cpool.tile([128, IMGS * KPJ * D], f32)
            nc.sync.dma_start(w2_t[:], w2)
            inv4_t = cpool.tile([4, W], f32)
            nc.sync.dma_start(inv4_t[:], inv4)
            vrow_t = cpool.tile([1, W], f32)
            nc.sync.dma_start(vrow_t[:], vrow)
            hmask_t = cpool.tile([NPERS, W], f32)
            nc.sync.dma_start(hmask_t[:], hmask)
            scales_t = cpool.tile([1, 2 * IMGS], f32)
            nc.sync.dma_start(scales_t[:], scales)
            ones4_t = cpool.tile([4, 1], f32)
            nc.vector.memset(ones4_t[:], 1.0)
            ones30_t = cpool.tile([NPERS, 1], f32)
            nc.vector.memset(ones30_t[:], 1.0)
            ones128_t = cpool.tile([128, 1], f32)
            nc.vector.memset(ones128_t[:], 1.0)
            ones1_t = cpool.tile([1, NPERS], f32)
            nc.vector.memset(ones1_t[:], 1.0)
            cols_t = cpool.tile([128, IMGS], f32)
            res_t = cpool.tile([1, IMGS * 2], f32)
            meanT_t = mpool.tile([4, W], f32)
            sqm_t = mpool.tile([4, W], f32)
            n2m_t = mpool.tile([4, W], f32)
            qrow_t = mpool.tile([1, W], f32)
            qv_t = mpool.tile([1, W], f32)
            dall_p = ppool.tile([NPERS, (IMGS - 1) * NPERS], f32, space="PSUM", tag="dall")
            d3_p = ppool.tile([NPERS, NPERS], f32, space="PSUM", tag="d3")

            # per-image pipeline (hidden under the gather chain for b<3)
            for b in range(IMGS):
                c30 = slice(b * NPERS, (b + 1) * NPERS)
                sl16 = slice(b * KPJ * D, (b + 1) * KPJ * D)
                tm_t = wpool.tile([128, KPJ * D], f32, tag="tm")
                msum_p = ppool.tile([4, NPERS], f32, space="PSUM", tag="msum", bufs=2)
                if b < IMGS - 1:
                    nc.vector.tensor_mul(tm_t[:], g_ts[b][:], mvis_t[:, sl16])
                    for j in range(KPJ):
                        nc.tensor.matmul(
                            out=msum_p[:],
                            lhsT=tm_t[:, j * D:(j + 1) * D],
                            rhs=member_t[:, j * NPERS:(j + 1) * NPERS],
                            start=(j == 0),
                            stop=(j == KPJ - 1),
                        )
                else:
                    # last image: column-wise so each msum matmul fires as
                    # soon as its gather column lands
                    for j in range(KPJ):
                        cj = slice(j * D, (j + 1) * D)
                        nc.vector.tensor_mul(
                            tm_t[:, cj], g_ts[b][:, cj],
                            mvis_t[:, b * KPJ * D + j * D:
                                   b * KPJ * D + (j + 1) * D],
                        )
                        nc.tensor.matmul(
                            out=msum_p[:],
                            lhsT=tm_t[:, cj],
                            rhs=member_t[:, j * NPERS:(j + 1) * NPERS],
                            start=(j == 0),
                            stop=(j == KPJ - 1),
                        )
                # pull term 1 dot: sum_kp |tm|^2 * invcv(person)
                tmw_t = wpool.tile([128, KPJ * D], f32, tag="tmw", bufs=2)
                nc.vector.tensor_mul(tmw_t[:], tm_t[:], w2_t[:, sl16])
                ttro_t = wpool.tile([128, KPJ * D], f32, tag="ttro", bufs=2)
                nc.vector.tensor_mul(ttro_t[:], tm_t[:], tmw_t[:])
                nc.vector.reduce_sum(cols_t[:, b:b + 1], ttro_t[:], axis=X.X)
                # means, q, diff matmuls
                nc.vector.tensor_mul(meanT_t[:, c30], msum_p[:], inv4_t[:, c30])
                nc.vector.tensor_mul(sqm_t[:, c30], meanT_t[:, c30], meanT_t[:, c30])
                q_p = ppool.tile([1, NPERS], f32, space="PSUM", tag="q", bufs=2)
                nc.tensor.matmul(
                    out=q_p[:], lhsT=ones4_t[:], rhs=sqm_t[:, c30],
                    start=True, stop=True,
                )
                nc.vector.tensor_mul(qv_t[:, c30], q_p[:], vrow_t[:, c30])
                nc.vector.tensor_scalar_mul(n2m_t[:, c30], meanT_t[:, c30], -2.0)
                nc.vector.tensor_copy(qrow_t[:, c30], q_p[:])
                dst_p = d3_p[:] if b == IMGS - 1 else dall_p[:, c30]
                nc.tensor.matmul(
                    out=dst_p, lhsT=n2m_t[:, c30], rhs=meanT_t[:, c30],
                    start=True, stop=False,
                )
                nc.tensor.matmul(
                    out=dst_p, lhsT=qrow_t[:, c30], rhs=ones1_t[:],
                    start=False, stop=False,
                )
                nc.tensor.matmul(
                    out=dst_p, lhsT=ones1_t[:], rhs=qrow_t[:, c30],
                    start=False, stop=True,
                )

            # finale split in two pieces: images 0..2 as soon as their diff
            # matmuls are done (under the gather chain), image 3 in the tail
            e_t = mpool.tile([NPERS, W], f32)
            m2_t = mpool.tile([NPERS, W], f32)
            c_t = mpool.tile([NPERS, W], f32)
            prow_t = mpool.tile([NPERS, IMGS], f32)
            for lo, hi in ((0, IMGS - 1), (IMGS - 1, IMGS)):
                csl = slice(lo * NPERS, hi * NPERS)
                src_p = d3_p[:] if lo == IMGS - 1 else dall_p[:, csl]
                nc.scalar.activation(
                    e_t[:, csl], src_p,
                    mybir.ActivationFunctionType.Exp, bias=0.0, scale=-1.0,
                )
                nc.vector.scalar_tensor_tensor(
                    m2_t[:, csl], src_p, 0.0, hmask_t[:, csl],
                    op0=mybir.AluOpType.not_equal, op1=mybir.AluOpType.mult,
                )
                nc.vector.tensor_mul(c_t[:, csl], e_t[:, csl], m2_t[:, csl])
                nc.vector.reduce_sum(
                    prow_t[:, lo:hi],
                    c_t[:, csl].rearrange("p (i q) -> p i q", q=NPERS),
                    axis=X.X,
                )
            pt_p = ppool.tile([1, IMGS], f32, space="PSUM", tag="pt")
            nc.tensor.matmul(
                out=pt_p[:], lhsT=ones30_t[:], rhs=prow_t[:], start=True, stop=True
            )
            t1_p = ppool.tile([1, IMGS], f32, space="PSUM", tag="t1")
            nc.tensor.matmul(
                out=t1_p[:], lhsT=ones128_t[:], rhs=cols_t[:], start=True, stop=True
            )
            term2_t = mpool.tile([1, IMGS], f32)
            nc.vector.reduce_sum(
                term2_t[:], qv_t[:].rearrange("o (i p) -> o i p", p=NPERS), axis=X.X
            )
            pull4_t = mpool.tile([1, IMGS], f32)
            nc.vector.tensor_sub(pull4_t[:], t1_p[:], term2_t[:])
            r3 = res_t[:].rearrange("o (i t) -> o i t", t=2)
            nc.vector.tensor_mul(
                r3[:, :, 0:1],
                pt_p[:].rearrange("o (i u) -> o i u", u=1),
                scales_t[0:1, 0:IMGS].rearrange("o (i u) -> o i u", u=1),
            )
            nc.vector.tensor_mul(
                r3[:, :, 1:2],
                pull4_t[:].rearrange("o (i u) -> o i u", u=1),
                scales_t[0:1, IMGS:2 * IMGS].rearrange("o (i u) -> o i u", u=1),
            )
            nc.sync.dma_start(out, res_t[:])

    nc.compile()
    return nc


_NC_CACHE = None


def _get_nc():
    global _NC_CACHE
    if _NC_CACHE is None:
        _NC_CACHE = _build_nc()
    return _NC_CACHE


def _static_member() -> np.ndarray:
    member = np.zeros((128, KPJ * NPERS), dtype=np.float32)
    for p in range(128):
        for j in range(KPJ):
            sl = KPJ * p + j
            if sl < NFLAT:
                member[p, j * NPERS + (sl // NKP)] = 1.0
    return member


def _host_prep(tags: np.ndarray, keypoints: np.ndarray):
    """Build the per-core input maps. tags [B,N,D] f32, keypoints [B,30,17,2]."""
    kp_idx = keypoints[..., 0].reshape(B, NFLAT).astype(np.int64)
    kp_vis = (keypoints[..., 1] > 0).reshape(B, NFLAT)
    upper = np.triu(np.ones((NPERS, NPERS), dtype=bool), 1)
    slot_person_full = np.full(128 * KPJ, -1, dtype=np.int64)
    slot_person_full[:NFLAT] = np.arange(NFLAT) // NKP

    in_maps = []
    for c in range(NCORES):
        tags_flat = np.ascontiguousarray(
            tags[c * IMGS:(c + 1) * IMGS].reshape(IMGS * N, D), dtype=np.float32
        )
        idx = np.zeros((128, IMGS * KPJ), dtype=np.int32)
        w2 = np.zeros((128, IMGS * KPJ * D), dtype=np.float32)
        mvis = np.zeros((128, IMGS * KPJ * D), dtype=np.float32)
        inv4 = np.zeros((4, W), dtype=np.float32)
        vrow = np.zeros((1, W), dtype=np.float32)
        hmask = np.zeros((NPERS, W), dtype=np.float32)
        scales = np.zeros((1, 2 * IMGS), dtype=np.float32)
        for lb in range(IMGS):
            gb = c * IMGS + lb
            fidx = kp_idx[gb]            # [510]
            fvis = kp_vis[gb]            # [510]
            slot_idx = np.zeros(128 * KPJ, dtype=np.int64)
            slot_vis = np.zeros(128 * KPJ, dtype=np.float32)
            slot_idx[:NFLAT] = fidx + lb * N
            slot_vis[:NFLAT] = fvis.astype(np.float32)
            idx[:, lb * KPJ:(lb + 1) * KPJ] = slot_idx.reshape(128, KPJ)
            vis_pk = fvis.reshape(NPERS, NKP)
            cnt = vis_pk.sum(axis=1).astype(np.float32)
            valid = cnt > 0
            safe_cnt = np.maximum(cnt, 1.0)
            invcv = valid / safe_cnt
            mvis[:, lb * KPJ * D:(lb + 1) * KPJ * D] = np.repeat(
                slot_vis.reshape(128, KPJ), D, axis=1
            )
            slot_w2 = np.zeros(128 * KPJ, dtype=np.float32)
            okf = slot_person_full >= 0
            slot_w2[okf] = invcv[slot_person_full[okf]]
            w2[:, lb * KPJ * D:(lb + 1) * KPJ * D] = np.repeat(
                slot_w2.reshape(128, KPJ), D, axis=1
            )
            inv4[:, lb * NPERS:(lb + 1) * NPERS] = (1.0 / safe_cnt)[None, :]
            vrow[0, lb * NPERS:(lb + 1) * NPERS] = valid
            hmask[:, lb * NPERS:(lb + 1) * NPERS] = (
                upper & valid[:, None] & valid[None, :]
            ).astype(np.float32)
            n = valid.sum().astype(np.float32)
            scales[0, lb] = 1.0 / ((n - 1.0) * n + EPS)
            scales[0, IMGS + lb] = 1.0 / (n + EPS)
        member = _static_member()
        in_maps.append(
            {
                "tags": tags_flat,
                "idx": idx,
                "mvis": mvis,
                "w2": w2,
                "member": member,
                "inv4": inv4,
                "vrow": vrow,
                "hmask": hmask,
                "scales": scales,
            }
        )
    return in_maps


def kernel(tags: np.ndarray, keypoints: np.ndarray) -> np.ndarray:
    tags = np.asarray(tags, dtype=np.float32)
    keypoints = np.asarray(keypoints)
    nc = _get_nc()
    in_maps = _host_prep(tags, keypoints)
    res = run_bass_kernel_spmd(nc, in_maps, core_ids=list(range(NCORES)))
    outs = [np.asarray(r["out"]).reshape(IMGS, 2) for r in res.results]
    return np.concatenate(outs, axis=0)
